# revision 18
# baseline (speedup 1.0000x reference)
"""FP8 GEMM kernel (MixLinear) for 8 trn2 NeuronCores.

Reference computation:
    s      = max(|x|) / 448                        (global fp32 scalar)
    q_x    = e4m3fn(clip(x / s, +-448))            (OCP e4m3fn)
    q_w    = e4m3fn(clip(w, +-448))                (scale_weight = 1)
    y      = (q_x @ q_w.T) * s + bias              (fp32 accum -> fp16)

Strategy: data-parallel over the 16384 token rows (2048 rows per core).
Host does layout + the static weight quantization (q_w is scale-1
e4m3fn rounding, bit-identical to the reference's static path -- in the
real workload the checkpoint ships pre-quantized fp8 weights).  Device
does the dynamic part: abs-max of x, a cross-core exchange of the
per-core maxima, activation quantization, DoubleRow fp8 matmul and
scale+bias eviction.

Critical-path design (v1 spent 103us before the first matmul):
  * all input DMA on the sync HWDGE ring, x tiles strictly before the
    fp8 weight tiles -> x (8 MiB) lands in ~24us at the full per-core
    HBM rate instead of sharing with w.
  * amax reduces output [P,2] fp16 slices so the DVE 2x 16-bit perf
    mode triggers (a [P,1] output forces 1x mode: 4.3us/tile, which
    made amax itself the critical path in earlier versions).
  * the global max is exchanged with one 64B AllGather.  The ncfw
    collective path has a fixed ~60us cold-start on this runtime
    (first doorbell pickup lands at ~61-68us wall no matter when the
    doorbell rings), so the schedule posts the local max well before
    the pickup, which removes the straggler waits the v1 kernel paid
    inside the collective.  (A direct remote-DMA exchange was tried:
    correct, but some deliveries are ~3ms-delayed on this runtime.)
  * the tiny cc bounce transfers ride the otherwise-idle scalar HWDGE
    ring so they never queue behind bulk weight DMA.

TRN e4m3 tops out at 240 (vs OCP 448), so x is quantized at half scale:
    q_half = trn_e4m3(x * (224/gmax))  ==  ocp_e4m3(x / s) / 2
exactly for all magnitudes >= 2^-6 * s (below that the two grids differ
by one subnormal bit -- negligible).  Weights (|w| <= 1/sqrt(2048)) are
in the range where the TRN and OCP grids agree exactly, so the host
e4m3fn bits are interpreted identically by the PE.  The output scale
is then 2*s = gmax/224.

DoubleRow pairing: adjacent d_in rows (2p, 2p+1) share a PE cell, so
each SBUF partition p loads one contiguous block of the transposed
operand -- max-rate DMA.
"""

import numpy as np

B, S, D_IN, D_OUT = 2, 8192, 2048, 2048
N_CORES = 8
TOK = B * S                  # 16384
TOK_PC = TOK // N_CORES      # 2048 token rows per core
P = 128
KP = D_IN // (2 * P)         # 8 k-pairs of 256 (DoubleRow granularity)
MT = TOK_PC // P             # 16 token tiles per core
N_TILE = 512
NT = D_OUT // N_TILE         # 4 output column tiles

_compiled = None


def _build():
    import concourse.bacc as bacc
    import concourse.tile as tile
    from concourse import mybir
    from concourse.masks import make_identity

    f16 = mybir.dt.float16
    f32 = mybir.dt.float32
    f8 = mybir.dt.float8e4
    Alu = mybir.AluOpType
    Axis = mybir.AxisListType
    Act = mybir.ActivationFunctionType

    nc = bacc.Bacc("TRN2", target_bir_lowering=False, debug=False,
                   num_devices=N_CORES)

    # xt: x^T shard, DRSW-interleaved on host: row (j*128+p) holds, for
    # each 128-token block mt with tokens m DESCENDING, the pair
    # [x(k=2jP+2p, m), x(k=2jP+2p+1, m)] -- the SW-interleaved stationary
    # layout the PE expects for DoubleRowSwInterleave (contiguous
    # LDWEIGHTS reads, unlike hardware DoubleRow's strided interleave).
    xt = nc.dram_tensor("xt", [KP * P, MT * P * 2], f16, kind="ExternalInput")
    wq = nc.dram_tensor("wq", [D_IN, D_OUT], f8, kind="ExternalInput")
    bias = nc.dram_tensor("bias", [D_OUT], f16, kind="ExternalInput")
    y = nc.dram_tensor("y", [TOK_PC, D_OUT], f16, kind="ExternalOutput")

    # DRAM bounce buffers for the max AllGather (16 f32 = 64B aligned)
    cc_in = nc.dram_tensor("cc_in", [16], f32)
    cc_out = nc.dram_tensor("cc_out", [16 * N_CORES], f32, addr_space="Shared")

    groups = [list(range(N_CORES))]

    with tile.TileContext(nc) as tc:
        with (
            tc.tile_pool(name="xpool", bufs=KP) as xpool,
            tc.tile_pool(name="qxpool", bufs=KP) as qxpool,
            tc.tile_pool(name="qwpool", bufs=KP) as qwpool,
            tc.tile_pool(name="small", bufs=1) as small,
            tc.tile_pool(name="ypool", bufs=3) as ypool,
            tc.tile_pool(name="psum", bufs=8, space="PSUM") as psum,
        ):
            # identity for the PE-transpose partition fold (gpsimd, instant)
            ident = small.tile([P, P], f32)
            make_identity(nc, ident[:])

            # ---- sync HWDGE ring, in priority order: bias, x, w ----
            bias_row = small.tile([1, D_OUT], f16)
            nc.sync.dma_start(bias_row[:], bias[None, :])

            x_sb = []
            for j in range(KP):
                t = xpool.tile([P, MT, P, 2], f16, tag="xsb")
                src = xt[j * P:(j + 1) * P, :]
                nc.sync.dma_start(
                    t[:], src.rearrange("p (a b t) -> p a b t", b=P, t=2))
                x_sb.append(t)

            qw = []
            for j in range(KP):
                qt = qwpool.tile([P, 2, D_OUT], f8, tag="qw")
                src = wq[2 * j * P:(2 * j + 2) * P, :]
                nc.sync.dma_start(qt[:], src.rearrange("(p t) n -> p t n", t=2))
                qw.append(qt)

            # ---- abs-max chases the x DMA ----
            pmax = small.tile([P, KP], f16)
            for j in range(KP):
                nc.vector.tensor_reduce(
                    out=pmax[:, j:j + 1], in_=x_sb[j][:], axis=Axis.XYZ,
                    op=Alu.max, apply_absolute_value=True)

            lmax = small.tile([P, 1], f32)
            nc.vector.tensor_reduce(out=lmax[:], in_=pmax[:], axis=Axis.X,
                                    op=Alu.max)
            # fold 128 partitions -> [1, 128] via PE transpose, then reduce
            lmax_t = psum.tile([1, P], f32, tag="ps", name="lmaxt")
            nc.tensor.transpose(lmax_t[:], lmax[:], ident[:])
            lmax16 = small.tile([1, 16], f32)
            nc.vector.memset(lmax16[:], 0.0)
            nc.vector.tensor_reduce(out=lmax16[:, 0:1], in_=lmax_t[:],
                                    axis=Axis.X, op=Alu.max)

            # ---- gather per-core maxima via AllGather ----
            # cc_in/gall ride the (otherwise idle) scalar HWDGE ring so the
            # tiny transfers never queue behind the weight DMA.  The ncfw
            # collective path has a fixed ~60us cold-start on this runtime
            # (measured: first doorbell pickup at ~61-68us wall regardless
            # of trigger time; direct remote-DMA exchange was tried and its
            # delivery is partially ~3ms-delayed here), so the kernel is
            # structured to have everything but the scale chain done before
            # the collective completes.
            nc.scalar.dma_start(cc_in[:], lmax16[:])
            nc.gpsimd.collective_compute(
                "AllGather", Alu.bypass, replica_groups=groups,
                ins=[cc_in.ap().opt()], outs=[cc_out.ap().opt()])
            gall = small.tile([1, 16 * N_CORES], f32)
            nc.scalar.dma_start(gall[:], cc_out[None, :])
            gmax0 = small.tile([1, 1], f32)
            nc.vector.tensor_reduce(out=gmax0[:], in_=gall[:], axis=Axis.X,
                                    op=Alu.max)
            # scale math on partition 0: col0 = inv_half, col1 = out_scale
            sc = small.tile([1, 2], f32)
            nc.vector.reciprocal(sc[:, 0:1], gmax0[:])
            nc.vector.tensor_scalar_mul(sc[:, 0:1], sc[:, 0:1], 224.0)
            nc.vector.tensor_scalar_mul(sc[:, 1:2], gmax0[:], 1.0 / 224.0)
            scales = small.tile([P, 2], f32)
            nc.gpsimd.partition_broadcast(scales[:], sc[:], P)
            inv_half = scales[:, 0:1]
            out_scale = scales[:, 1:2]

            # bias broadcast to all partitions (gpsimd, off critical path)
            bias_bc = small.tile([P, D_OUT], f16)
            nc.gpsimd.partition_broadcast(bias_bc[:], bias_row[:], P)

            # ---- quantize x at half scale ----
            # first 2 token tiles of every k-pair go first (on DVE) so the
            # matmul phase can start while the rest quantizes (DVE/ACT split)
            qx = []
            for j in range(KP):
                qt = qxpool.tile([P, MT, P, 2], f8, tag="qx")
                nc.vector.tensor_scalar(out=qt[:, 0:2, :, :],
                                        in0=x_sb[j][:, 0:2, :, :],
                                        scalar1=inv_half[:, 0:1],
                                        scalar2=None, op0=Alu.mult)
                qx.append(qt)
            for j in range(KP):
                if j % 2 == 0:
                    nc.vector.tensor_scalar(out=qx[j][:, 2:, :, :],
                                            in0=x_sb[j][:, 2:, :, :],
                                            scalar1=inv_half[:, 0:1],
                                            scalar2=None, op0=Alu.mult)
                else:
                    nc.scalar.activation(qx[j][:, 2:, :, :],
                                         x_sb[j][:, 2:, :, :],
                                         Act.Copy, scale=inv_half[:, 0:1])

            # ---- DoubleRow fp8 matmul + fused scale/bias ----
            for mt in range(MT):
                ps = [psum.tile([P, N_TILE], f32, tag="ps", name=f"ps{nt}")
                      for nt in range(NT)]
                for j in range(KP):
                    lhsT = qx[j][:, mt, :, :]
                    for nt in range(NT):
                        nc.tensor.matmul(
                            ps[nt][:],
                            lhsT,
                            qw[j][:, :, nt * N_TILE:(nt + 1) * N_TILE],
                            start=(j == 0), stop=(j == KP - 1),
                            perf_mode=mybir.MatmulPerfMode.DoubleRowSwInterleave)
                ysb = ypool.tile([P, D_OUT], f16, tag="ysb")
                for nt in range(NT):
                    nc.vector.scalar_tensor_tensor(
                        out=ysb[:, nt * N_TILE:(nt + 1) * N_TILE],
                        in0=ps[nt][:], scalar=out_scale[:, 0:1],
                        in1=bias_bc[:, nt * N_TILE:(nt + 1) * N_TILE],
                        op0=Alu.mult, op1=Alu.add)
                    nc.sync.dma_start(
                        y[mt * P:(mt + 1) * P, nt * N_TILE:(nt + 1) * N_TILE],
                        ysb[:, nt * N_TILE:(nt + 1) * N_TILE])

    nc.compile()
    return nc


def _get_compiled():
    global _compiled
    if _compiled is None:
        _compiled = _build()
    return _compiled


def _quant_weight_host(weight):
    """Static e4m3fn quantization of w at scale 1 (matches reference
    _quant_fp8(weight, 1.0) bit-exactly: same clip + RNE cast)."""
    import ml_dtypes
    q = np.clip(weight.astype(np.float32), -448.0, 448.0)
    return q.astype(ml_dtypes.float8_e4m3fn)


def run(x, weight, bias, **kw):
    """Shard + run on 8 cores; returns (full_output, BassKernelResults)."""
    from concourse.bass_utils import run_bass_kernel_spmd

    nc = _get_compiled()

    x = np.asarray(x, dtype=np.float16)
    weight = np.asarray(weight, dtype=np.float16)
    bias = np.asarray(bias, dtype=np.float16)
    xr = x.reshape(TOK, D_IN)
    wqt = np.ascontiguousarray(_quant_weight_host(weight).T)   # [d_in, d_out]
    in_maps = []
    for i in range(N_CORES):
        shard = xr[i * TOK_PC:(i + 1) * TOK_PC]                # [tok_pc, d_in]
        t = shard.T.reshape(KP, P, 2, MT, P)                   # [j, p, i, mt, m]
        t = t[:, :, :, :, ::-1]                                # m descending
        t = t.transpose(0, 1, 3, 4, 2)                         # [j, p, mt, m, i]
        xt_i = np.ascontiguousarray(t.reshape(KP * P, MT * P * 2))
        in_maps.append({
            "xt": xt_i,
            "wq": wqt,
            "bias": bias,
        })
    res = run_bass_kernel_spmd(nc, in_maps, core_ids=list(range(N_CORES)), **kw)
    out = np.concatenate([res.results[i]["y"] for i in range(N_CORES)], axis=0)
    return out.reshape(B, S, D_OUT), res


def kernel(x, weight, bias):
    out, _ = run(x, weight, bias)
    return out


# revision 20
# speedup vs baseline: 1.0651x; 1.0651x over previous
"""FP8 GEMM kernel (MixLinear) for 8 trn2 NeuronCores.

Reference computation:
    s      = max(|x|) / 448                        (global fp32 scalar)
    q_x    = e4m3fn(clip(x / s, +-448))            (OCP e4m3fn)
    q_w    = e4m3fn(clip(w, +-448))                (scale_weight = 1)
    y      = (q_x @ q_w.T) * s + bias              (fp32 accum -> fp16)

Strategy: data-parallel over the 16384 token rows (2048 rows per core).
Host does layout + the static weight quantization (q_w is scale-1
e4m3fn rounding, bit-identical to the reference's static path -- in the
real workload the checkpoint ships pre-quantized fp8 weights).  Device
does the dynamic part: abs-max of x, a cross-core exchange of the
per-core maxima, activation quantization, DoubleRow fp8 matmul and
scale+bias eviction.

Critical-path design (v1 spent 103us before the first matmul):
  * all input DMA on the sync HWDGE ring, x tiles strictly before the
    fp8 weight tiles -> x (8 MiB) lands in ~24us at the full per-core
    HBM rate instead of sharing with w.
  * amax reduces output [P,2] fp16 slices so the DVE 2x 16-bit perf
    mode triggers (a [P,1] output forces 1x mode: 4.3us/tile, which
    made amax itself the critical path in earlier versions).
  * the global max is exchanged with one 64B AllGather.  The ncfw
    collective path has a fixed ~60us cold-start on this runtime
    (first doorbell pickup lands at ~61-68us wall no matter when the
    doorbell rings), so the schedule posts the local max well before
    the pickup, which removes the straggler waits the v1 kernel paid
    inside the collective.  (A direct remote-DMA exchange was tried:
    correct, but some deliveries are ~3ms-delayed on this runtime.)
  * the tiny cc bounce transfers ride the otherwise-idle scalar HWDGE
    ring so they never queue behind bulk weight DMA.

TRN e4m3 tops out at 240 (vs OCP 448), so x is quantized at half scale:
    q_half = trn_e4m3(x * (224/gmax))  ==  ocp_e4m3(x / s) / 2
exactly for all magnitudes >= 2^-6 * s (below that the two grids differ
by one subnormal bit -- negligible).  Weights (|w| <= 1/sqrt(2048)) are
in the range where the TRN and OCP grids agree exactly, so the host
e4m3fn bits are interpreted identically by the PE.  The output scale
is then 2*s = gmax/224.

DoubleRow pairing: adjacent d_in rows (2p, 2p+1) share a PE cell, so
each SBUF partition p loads one contiguous block of the transposed
operand -- max-rate DMA.
"""

import numpy as np

B, S, D_IN, D_OUT = 2, 8192, 2048, 2048
N_CORES = 8
TOK = B * S                  # 16384
TOK_PC = TOK // N_CORES      # 2048 token rows per core
P = 128
KP = D_IN // (2 * P)         # 8 k-pairs of 256 (DoubleRow granularity)
MT = TOK_PC // P             # 16 token tiles per core
N_TILE = 512
NT = D_OUT // N_TILE         # 4 output column tiles

_compiled = None


def _build():
    import concourse.bacc as bacc
    import concourse.tile as tile
    from concourse import mybir
    from concourse.masks import make_identity

    f16 = mybir.dt.float16
    f32 = mybir.dt.float32
    f8 = mybir.dt.float8e4
    Alu = mybir.AluOpType
    Axis = mybir.AxisListType
    Act = mybir.ActivationFunctionType

    nc = bacc.Bacc("TRN2", target_bir_lowering=False, debug=False,
                   num_devices=N_CORES)

    # xt: x^T shard [d_in, tok_pc]; wq: w^T [d_in, d_out] fp8 (replicated)
    xt = nc.dram_tensor("xt", [D_IN, TOK_PC], f16, kind="ExternalInput")
    # wq: w^T with k-pairs interleaved per column on host -- row r holds
    # [w(k=2r, n), w(k=2r+1, n)] adjacent, so the DoubleRow moving-operand
    # AP [P, 2, N] (strides 1, 2) streams one contiguous run per partition
    # instead of two 512B runs.
    wq = nc.dram_tensor("wq", [KP * P, D_OUT * 2], f8, kind="ExternalInput")
    bias = nc.dram_tensor("bias", [D_OUT], f16, kind="ExternalInput")
    y = nc.dram_tensor("y", [TOK_PC, D_OUT], f16, kind="ExternalOutput")

    # DRAM bounce buffers for the max AllGather (16 f32 = 64B aligned)
    cc_in = nc.dram_tensor("cc_in", [16], f32)
    cc_out = nc.dram_tensor("cc_out", [16 * N_CORES], f32, addr_space="Shared")

    groups = [list(range(N_CORES))]

    with tile.TileContext(nc) as tc:
        with (
            tc.tile_pool(name="xpool", bufs=KP) as xpool,
            tc.tile_pool(name="qxpool", bufs=KP) as qxpool,
            tc.tile_pool(name="qwpool", bufs=KP) as qwpool,
            tc.tile_pool(name="small", bufs=1) as small,
            tc.tile_pool(name="ypool", bufs=3) as ypool,
            tc.tile_pool(name="psum", bufs=8, space="PSUM") as psum,
        ):
            # identity for the PE-transpose partition fold (gpsimd, instant)
            ident = small.tile([P, P], f32)
            make_identity(nc, ident[:])

            # ---- sync HWDGE ring, in priority order: bias, x, w ----
            bias_row = small.tile([1, D_OUT], f16)
            nc.sync.dma_start(bias_row[:], bias[None, :])

            x_sb = []
            for j in range(KP):
                t = xpool.tile([P, 2, TOK_PC], f16, tag="xsb")
                src = xt[2 * j * P:(2 * j + 2) * P, :]
                nc.sync.dma_start(t[:], src.rearrange("(p t) m -> p t m", t=2))
                x_sb.append(t)

            qw = []
            for j in range(KP):
                qt = qwpool.tile([P, D_OUT, 2], f8, tag="qw")
                src = wq[j * P:(j + 1) * P, :]
                nc.sync.dma_start(qt[:], src.rearrange("p (n t) -> p n t", t=2))
                qw.append(qt)

            # ---- abs-max chases the x DMA ----
            # [P,2] fp16 output slices keep the DVE in 2x 16-bit mode.
            pmax = small.tile([P, 2 * KP], f16)
            for j in range(KP):
                nc.vector.tensor_reduce(
                    out=pmax[:, 2 * j:2 * j + 2], in_=x_sb[j][:], axis=Axis.X,
                    op=Alu.max, apply_absolute_value=True)

            lmax = small.tile([P, 1], f32)
            nc.vector.tensor_reduce(out=lmax[:], in_=pmax[:], axis=Axis.X,
                                    op=Alu.max)
            # fold 128 partitions -> [1, 128] via PE transpose, then reduce
            lmax_t = psum.tile([1, P], f32, tag="ps", name="lmaxt")
            nc.tensor.transpose(lmax_t[:], lmax[:], ident[:])
            lmax16 = small.tile([1, 16], f32)
            nc.vector.memset(lmax16[:], 0.0)
            nc.vector.tensor_reduce(out=lmax16[:, 0:1], in_=lmax_t[:],
                                    axis=Axis.X, op=Alu.max)

            # ---- gather per-core maxima via AllGather ----
            # cc_in/gall ride the (otherwise idle) scalar HWDGE ring so the
            # tiny transfers never queue behind the weight DMA.  The ncfw
            # collective path has a fixed ~60us cold-start on this runtime
            # (measured: first doorbell pickup at ~61-68us wall regardless
            # of trigger time; direct remote-DMA exchange was tried and its
            # delivery is partially ~3ms-delayed here), so the kernel is
            # structured to have everything but the scale chain done before
            # the collective completes.
            nc.scalar.dma_start(cc_in[:], lmax16[:])
            nc.gpsimd.collective_compute(
                "AllGather", Alu.bypass, replica_groups=groups,
                ins=[cc_in.ap().opt()], outs=[cc_out.ap().opt()])
            gall = small.tile([1, 16 * N_CORES], f32)
            nc.scalar.dma_start(gall[:], cc_out[None, :])
            gmax0 = small.tile([1, 1], f32)
            nc.vector.tensor_reduce(out=gmax0[:], in_=gall[:], axis=Axis.X,
                                    op=Alu.max)
            # scale math on partition 0: col0 = inv_half, col1 = out_scale
            sc = small.tile([1, 2], f32)
            nc.vector.reciprocal(sc[:, 0:1], gmax0[:])
            nc.vector.tensor_scalar_mul(sc[:, 0:1], sc[:, 0:1], 224.0)
            nc.vector.tensor_scalar_mul(sc[:, 1:2], gmax0[:], 1.0 / 224.0)
            scales = small.tile([P, 2], f32)
            nc.gpsimd.partition_broadcast(scales[:], sc[:], P)
            inv_half = scales[:, 0:1]
            out_scale = scales[:, 1:2]

            # bias broadcast to all partitions (gpsimd, off critical path)
            bias_bc = small.tile([P, D_OUT], f16)
            nc.gpsimd.partition_broadcast(bias_bc[:], bias_row[:], P)

            # ---- quantize x at half scale ----
            # first 2 token tiles of every k-pair go first (on DVE) so the
            # matmul phase can start while the rest quantizes (DVE/ACT split)
            C0 = P
            qx = []
            for j in range(KP):
                qt = qxpool.tile([P, 2, TOK_PC], f8, tag="qx")
                if j % 2 == 0:
                    nc.vector.tensor_scalar(out=qt[:, :, :C0],
                                            in0=x_sb[j][:, :, :C0],
                                            scalar1=inv_half[:, 0:1],
                                            scalar2=None, op0=Alu.mult)
                else:
                    nc.scalar.activation(qt[:, :, :C0], x_sb[j][:, :, :C0],
                                         Act.Copy, scale=inv_half[:, 0:1])
                qx.append(qt)
            for j in range(KP):
                if j % 2 == 0:
                    nc.vector.tensor_scalar(out=qx[j][:, :, C0:],
                                            in0=x_sb[j][:, :, C0:],
                                            scalar1=inv_half[:, 0:1],
                                            scalar2=None, op0=Alu.mult)
                else:
                    nc.scalar.activation(qx[j][:, :, C0:], x_sb[j][:, :, C0:],
                                         Act.Copy, scale=inv_half[:, 0:1])

            # ---- DoubleRow fp8 matmul + fused scale/bias ----
            for mt in range(MT):
                ps = [psum.tile([P, N_TILE], f32, tag="ps", name=f"ps{nt}")
                      for nt in range(NT)]
                for j in range(KP):
                    lhsT = qx[j][:, :, mt * P:(mt + 1) * P]
                    for nt in range(NT):
                        rhs = qw[j][:, nt * N_TILE:(nt + 1) * N_TILE, :]
                        nc.tensor.matmul(
                            ps[nt][:],
                            lhsT,
                            rhs.rearrange("p n t -> p t n"),
                            start=(j == 0), stop=(j == KP - 1),
                            perf_mode=mybir.MatmulPerfMode.DoubleRow)
                ysb = ypool.tile([P, D_OUT], f16, tag="ysb")
                for nt in range(NT):
                    nc.vector.scalar_tensor_tensor(
                        out=ysb[:, nt * N_TILE:(nt + 1) * N_TILE],
                        in0=ps[nt][:], scalar=out_scale[:, 0:1],
                        in1=bias_bc[:, nt * N_TILE:(nt + 1) * N_TILE],
                        op0=Alu.mult, op1=Alu.add)
                    nc.sync.dma_start(
                        y[mt * P:(mt + 1) * P, nt * N_TILE:(nt + 1) * N_TILE],
                        ysb[:, nt * N_TILE:(nt + 1) * N_TILE])

    nc.compile()
    return nc


def _get_compiled():
    global _compiled
    if _compiled is None:
        _compiled = _build()
    return _compiled


def _quant_weight_host(weight):
    """Static e4m3fn quantization of w at scale 1 (matches reference
    _quant_fp8(weight, 1.0) bit-exactly: same clip + RNE cast)."""
    import ml_dtypes
    q = np.clip(weight.astype(np.float32), -448.0, 448.0)
    return q.astype(ml_dtypes.float8_e4m3fn)


def run(x, weight, bias, **kw):
    """Shard + run on 8 cores; returns (full_output, BassKernelResults)."""
    from concourse.bass_utils import run_bass_kernel_spmd

    nc = _get_compiled()

    x = np.asarray(x, dtype=np.float16)
    weight = np.asarray(weight, dtype=np.float16)
    bias = np.asarray(bias, dtype=np.float16)
    xt = np.ascontiguousarray(x.reshape(TOK, D_IN).T)          # [d_in, tok]
    wt = _quant_weight_host(weight).T                          # [d_in, d_out]
    wqt = np.ascontiguousarray(
        wt.reshape(KP * P, 2, D_OUT).transpose(0, 2, 1).reshape(
            KP * P, D_OUT * 2))                                # pairs adjacent
    in_maps = []
    for i in range(N_CORES):
        in_maps.append({
            "xt": np.ascontiguousarray(xt[:, i * TOK_PC:(i + 1) * TOK_PC]),
            "wq": wqt,
            "bias": bias,
        })
    res = run_bass_kernel_spmd(nc, in_maps, core_ids=list(range(N_CORES)), **kw)
    out = np.concatenate([res.results[i]["y"] for i in range(N_CORES)], axis=0)
    return out.reshape(B, S, D_OUT), res


def kernel(x, weight, bias):
    out, _ = run(x, weight, bias)
    return out
